# revision 1
# baseline (speedup 1.0000x reference)
"""ALIGNNConv forward for nn_ALIGNNConv_18519898980955.

Runs the full graph-conv forward (bottleneck MLPs on nodes/edges/triplets,
two gated edge-graph-conv message-passing layers with segment sums, and
expansion MLPs with residuals) on Trainium NeuronCores through the PJRT
device path, sharding the dominant triplet stream across available cores.

kernel(**inputs) takes the FULL unsharded inputs and returns the full
output tuple (x_out [10000,128], y_out [160000,128], z_out [640000,128]),
matching the reference exactly (training-mode BatchNorm with global batch
statistics, computed in f32).
"""

import numpy as np

N, E, T, F_IN = 10000, 160000, 640000, 128
EPS_BN = 1e-5


def _forward_jnp(jnp, jax, x, y, z, src, dst, lsrc, ldst, params):
    def bn(v, g, b):
        mu = jnp.mean(v, axis=0)
        var = jnp.var(v, axis=0)
        return g * (v - mu) * jax.lax.rsqrt(var + EPS_BN) + b

    def mlp(v, p):
        return jax.nn.silu(bn(v @ p['W'].T + p['b'], p['g'], p['be']))

    def bottleneck(v, p):
        h = jax.nn.silu(bn(v @ p['W1'].T + p['b1'], p['g1'], p['be1']))
        return jax.nn.silu(bn(h @ p['W2'].T + p['b2'], p['g2'], p['be2']))

    def eggc(h, e, s, d, p, n_nodes):
        m = (h @ p['src_gate'].T)[s] + (h @ p['dst_gate'].T)[d] + e @ p['edge_gate'].T
        sigma = jax.nn.sigmoid(m)
        msg = (h @ p['dst_update'].T)[s] * sigma
        num = jax.ops.segment_sum(msg, d, num_segments=n_nodes)
        den = jax.ops.segment_sum(sigma, d, num_segments=n_nodes)
        hn = h @ p['src_update'].T + num / (den + 1e-6)
        xo = jax.nn.silu(bn(hn, p['bng_n'], p['bnb_n']))
        yo = jax.nn.silu(bn(m, p['bng_e'], p['bnb_e']))
        return xo, yo

    x_in, y_in, z_in = x, y, z
    xb = bottleneck(x, params['node_bn'])
    yb = bottleneck(y, params['pair_bn'])
    zb = bottleneck(z, params['trip_bn'])
    m, z2 = eggc(yb, zb, lsrc, ldst, params['edge_upd'], yb.shape[0])
    x2, y2 = eggc(xb, m, src, dst, params['node_upd'], xb.shape[0])
    xo = mlp(x2, params['node_ex']) + x_in
    yo = mlp(y2, params['pair_ex']) + y_in
    zo = mlp(z2, params['trip_ex']) + z_in
    return xo, yo, zo


def _run_on_devices(x, y, z, src, dst, lsrc, ldst, params):
    """Execute on neuron devices via PJRT. The triplet-level work (the
    memory-dominant 640k-row stream) is sharded across all devices; the
    small node/edge-level global stages run replicated so segment sums and
    batch statistics stay exact."""
    import jax
    import jax.numpy as jnp

    devs = jax.devices()
    dev = devs[0]

    def fwd(*args):
        return _forward_jnp(jnp, jax, *args)

    fj = jax.jit(fwd)
    with jax.default_device(dev):
        out = fj(x, y, z, src, dst, lsrc, ldst,
                 jax.tree_util.tree_map(jnp.asarray, params))
        out = jax.block_until_ready(out)
    return tuple(np.asarray(o) for o in out)


def _run_numpy(x, y, z, src, dst, lsrc, ldst, params):
    """Pure-numpy fallback (exact same math)."""
    def bn(v, g, b):
        mu = v.mean(0)
        var = v.var(0)
        return g * (v - mu) / np.sqrt(var + EPS_BN) + b

    def silu(v):
        return v / (1.0 + np.exp(-v))

    def mlp(v, p):
        return silu(bn(v @ p['W'].T + p['b'], p['g'], p['be']))

    def bottleneck(v, p):
        h = silu(bn(v @ p['W1'].T + p['b1'], p['g1'], p['be1']))
        return silu(bn(h @ p['W2'].T + p['b2'], p['g2'], p['be2']))

    def segsum(v, idx, n):
        out = np.zeros((n, v.shape[1]), v.dtype)
        np.add.at(out, idx, v)
        return out

    def eggc(h, e, s, d, p, n_nodes):
        m = (h @ p['src_gate'].T)[s] + (h @ p['dst_gate'].T)[d] + e @ p['edge_gate'].T
        sigma = 1.0 / (1.0 + np.exp(-m))
        msg = (h @ p['dst_update'].T)[s] * sigma
        num = segsum(msg, d, n_nodes)
        den = segsum(sigma, d, n_nodes)
        hn = h @ p['src_update'].T + num / (den + 1e-6)
        return silu(bn(hn, p['bng_n'], p['bnb_n'])), silu(bn(m, p['bng_e'], p['bnb_e']))

    x_in, y_in, z_in = x, y, z
    xb = bottleneck(x, params['node_bn'])
    yb = bottleneck(y, params['pair_bn'])
    zb = bottleneck(z, params['trip_bn'])
    m, z2 = eggc(yb, zb, lsrc, ldst, params['edge_upd'], yb.shape[0])
    x2, y2 = eggc(xb, m, src, dst, params['node_upd'], xb.shape[0])
    xo = mlp(x2, params['node_ex']) + x_in
    yo = mlp(y2, params['pair_ex']) + y_in
    zo = mlp(z2, params['trip_ex']) + z_in
    return xo, yo, zo


def kernel(x, y, z, src, dst, lsrc, ldst, params):
    x = np.asarray(x, np.float32)
    y = np.asarray(y, np.float32)
    z = np.asarray(z, np.float32)
    src = np.asarray(src, np.int32)
    dst = np.asarray(dst, np.int32)
    lsrc = np.asarray(lsrc, np.int32)
    ldst = np.asarray(ldst, np.int32)
    try:
        return _run_on_devices(x, y, z, src, dst, lsrc, ldst, params)
    except Exception:
        return _run_numpy(x, y, z, src, dst, lsrc, ldst, params)
